# revision 25
# baseline (speedup 1.0000x reference)
"""Cumulative-FFT Trainium2 kernel (multi-DMA slab stores, 249 us).

out[b,t,d,k,c] = pos_norm[t] * cumsum_t( x[b,t,d] * twiddles[t,k,c] )

Shapes (hardcoded): x (4,1024,512) bf16, twiddles (1024,32,2) bf16,
pos_norm (1024,) bf16  ->  out (4,1024,512,32,2) bf16.

Sharding: 8 cores = batch(4) x d_model-half(2). Each core computes a
(1024, 256*64) bf16 shard (32 MiB) -- data-parallel over B, tensor-parallel
over D, nothing crosses cores.

Per-core algorithm: the cumsum along t is done as a per-block triangular
matmul on the TensorEngine. t is split into blocks of 127 rows; the moving
operand c holds the bf16 contributions c[s, kc*256+d] = x[s,d]*tw[s,kc]
plus one extra row (s = L) holding the carry = column sums of all previous
blocks (maintained by a tiny tw^T @ x matmul per block). The stationary
operand folds the causal mask and the pos_norm[t] scale:

    utri[s, t] = pos_norm[t] * (1 if (s <= t or s == L) else 0)

so  psum[t, n] = pos[t] * (carry[n] + sum_{s<=t} c[s, n])  comes out of the
matmul fully finished.

DMA model (microbenched): ONE dma_start lands on very few SDMA engines
(~27 GB/s each); aggregate store bandwidth comes from many independent
DMAs in flight (up to ~15 engines, ~150-210 GB/s per core) and requires a
contiguous DRAM destination (strided column-group stores drop to ~4.5
GB/s/engine). Issue cost is ~0.6 us/dma on either DGE path. Each block's
4 MiB output is therefore stored as EIGHT contiguous 512 KB row-slabs
(16 rows x 32 KB -- 16-row slabs spread across all 16 engines; 8-row
slabs were observed to collapse onto engines 0-7 only), alternating the
sync/scalar HWDGE rings, with og triple-buffered so ~2-3 blocks of slabs
stay in flight. Stores are the wall: ~32 MiB/shard at ~150-200 GB/s.

All carries are precomputed up front (8 tiny PE delta matmuls + DVE
prefix adds into carry_sb), and each block's carry row is DMA'd into its
c tile ~one full block ahead of the matmuls that read it, so slab-store
queueing on the carry's SDMA engine never stalls the PE.

Engine split per block (2.08M build elems + 2.08M evict elems), all under
the ~23 us/block store cadence: DVE builds kc 0..DVE_KC-1 (4-D broadcast
multiply, ~240 G/s) + evicts groups 0..DVE_EG-1 (~107 G/s 1x, f32 PSUM
source); GPSIMD builds the remaining kc (slow, ~20-60 G/s -- its 4-D op
has ~6-9 us fixed overhead) ; ACT evicts the other 8 groups (~119 G/s)
and issues half the slabs; sync ring takes input preloads + 4 slabs.
"""

import sys

sys.path.insert(0, "/opt/trn_rl_repo")

import ml_dtypes
import numpy as np

import concourse.bass as bass
import concourse.mybir as mybir
import concourse.tile as tile
from concourse import bacc
import concourse.bass_utils as _bu
from concourse.bass_utils import run_bass_kernel_spmd

B, T, D = 4, 1024, 512
KC = 64            # 32 freqs x (cos,sin), flattened innermost dims of out
DSH = D // 2       # d-slice per core
NKC = DSH * KC     # free elements per t per core (16384)
BLK = 127          # data rows per t-block; row L is the carry row
NBLK = (T + BLK - 1) // BLK  # 9 (8 x 127 + 1 x 8)
XTW = DSH + KC     # columns of the packed x||tw input (320)

BF16 = mybir.dt.bfloat16
F32 = mybir.dt.float32

# groups of consecutive 512-wide matmul tiles evicted by one copy op
_EVICT_GROUPS = [(g * 4, 4) for g in range(8)]

# --- engine work-split knobs ---
DVE_KC = 58      # kc slices 0..DVE_KC-1 built on DVE, rest on GPSIMD
DVE_CHUNKS = [(0, 16), (16, 32), (32, 58)]  # build split, aligned to mm groups
DVE_EG = 1       # evict groups 0..DVE_EG-1 on DVE, rest on ACT
SLAB = 16        # rows per store slab (8 slabs per 127-row block)

LAST_RESULTS = None  # set by kernel(); test.py reads exec_time_ns from here


def _build_utri(pos_norm: np.ndarray) -> np.ndarray:
    """Stationary operands for all blocks, packed (128, NBLK*128) bf16."""
    pos = np.asarray(pos_norm).astype(np.float32)
    utri = np.zeros((128, NBLK * 128), np.float32)
    s = np.arange(128)[:, None]
    for k in range(NBLK):
        t0 = k * BLK
        L = min(BLK, T - t0)
        t = np.arange(L)[None, :]
        mask = ((s < L) & (s <= t)) | (s == L)
        utri[:, 128 * k : 128 * k + L] = mask * pos[t0 : t0 + L][None, :]
    return utri.astype(ml_dtypes.bfloat16)


def _build_program() -> bass.Bass:
    nc = bacc.Bacc("TRN2", target_bir_lowering=False, debug=False)
    xtw_d = nc.dram_tensor("xtw", [T, XTW], BF16, kind="ExternalInput").ap()
    utri_d = nc.dram_tensor("utri", [128, NBLK * 128], BF16, kind="ExternalInput").ap()
    out_d = nc.dram_tensor("out_shard", [T, NKC], BF16, kind="ExternalOutput").ap()

    with tile.TileContext(nc) as tc:
        with (
            tc.tile_pool(name="singles", bufs=1) as singles,
            tc.tile_pool(name="cp", bufs=2) as cp,
            tc.tile_pool(name="outp", bufs=3) as outp,
            tc.tile_pool(name="repp", bufs=2) as repp,
            tc.tile_pool(name="pmain", bufs=2, space="PSUM") as pmain,
        ):
            # block 0's inputs first (they gate the first build), then utri,
            # then the rest spread across BOTH HWDGE rings -- each dma_start
            # drains on one SDMA engine, so serializing 10 loads behind the
            # 288 KB utri load on one ring costs ~25 us of fill
            xtw_sb = singles.tile([128, NBLK * XTW], BF16)
            utri_sb = singles.tile([128, NBLK * 128], BF16)
            nc.sync.dma_start(
                out=xtw_sb[:BLK, 0:XTW], in_=xtw_d[0:BLK, :]
            )
            nc.scalar.dma_start(out=utri_sb[:, :], in_=utri_d[:, :])
            for k in range(1, NBLK):
                t0 = k * BLK
                L = min(BLK, T - t0)
                eng = nc.sync if k % 2 == 1 else nc.scalar
                eng.dma_start(
                    out=xtw_sb[:L, k * XTW : (k + 1) * XTW],
                    in_=xtw_d[t0 : t0 + L, :],
                )
            # back-to-back dummy matmuls (~6us cold): trips the PE HAM activity
            # monitor so the real matmuls run at 2.4 GHz instead of 1.2
            for _ in range(14):
                warm_ps = pmain.tile([128, 2048], F32, name="pg")
                nc.tensor.matmul(
                    warm_ps[:KC, :DSH],
                    lhsT=utri_sb[:128, 0:KC],
                    rhs=utri_sb[:128, 0:DSH],
                    start=True, stop=True,
                )

            def build(eng, c_sb, L, a0, a1, x_sl, rep16):
                na = a1 - a0
                c_v = c_sb[:L, a0 * DSH : a1 * DSH].rearrange(
                    "p (a b c) -> p a b c", b=16, c=16
                )
                x_v = (
                    x_sl.rearrange("p (b c) -> p b c", c=16)
                    .unsqueeze(1)
                    .broadcast_to((L, na, 16, 16))
                )
                rep_v = (
                    rep16[:L, a0 * 16 : a1 * 16]
                    .rearrange("p (a c) -> p a c", c=16)
                    .unsqueeze(2)
                    .broadcast_to((L, na, 16, 16))
                )
                eng.tensor_mul(c_v, x_v, rep_v)

            # all carries precomputed up front: carry_sb[:, 256k:256(k+1)]
            # = sum_{j<k} tw_j^T @ x_j   (kc-major, matches the c layout)
            carry_sb = singles.tile([KC, NBLK * DSH], BF16)
            nc.vector.memset(carry_sb[:, 0:DSH], 0.0)

            csb = [None, None]
            ogb = [None, None, None]
            store_pending = None  # (k, t0, L) awaiting emission

            gps_slabs = []  # deferred (t0, s0, s1, og) on the GPS queue

            def emit_stores():
                # sync/scalar slabs issue immediately (HWDGE reaches SDMA
                # engines 0-7 only); gpsimd slabs (engines 8-15 too) are
                # stashed and emitted after the NEXT block's GPS build so
                # their evict-sem waits never stall the in-order GPS queue
                nonlocal store_pending
                if store_pending is None:
                    return
                pk, pt0, pL = store_pending
                og = ogb[pk % 3]
                s0 = 0
                si = 0
                engs = [None, nc.sync, nc.sync, nc.sync,
                        None, nc.scalar, nc.sync, nc.scalar]
                while s0 < pL:
                    s1 = min(s0 + SLAB, pL)
                    eng = engs[si % 8]
                    if eng is None:
                        gps_slabs.append((pt0, s0, s1, og))
                    else:
                        eng.dma_start(
                            out=out_d[pt0 + s0 : pt0 + s1, :], in_=og[s0:s1, :]
                        )
                    si += 1
                    s0 = s1
                store_pending = None

            def emit_gps_slabs():
                for pt0, s0, s1, og in gps_slabs:
                    nc.gpsimd.dma_start(
                        out=out_d[pt0 + s0 : pt0 + s1, :], in_=og[s0:s1, :]
                    )
                gps_slabs.clear()

            def carry_dma(k):
                # carry row of block k, prefetched ~one block ahead of its
                # matmuls so slab-store queueing on its SDMA engine is hidden
                Lk = min(BLK, T - k * BLK)
                nc.gpsimd.dma_start(
                    out=csb[k % 2][Lk : Lk + 1, :],
                    in_=carry_sb[:, k * DSH : (k + 1) * DSH],
                )

            for k in range(NBLK):
                t0 = k * BLK
                L = min(BLK, T - t0)
                x_sl = xtw_sb[:L, k * XTW : k * XTW + DSH]
                tw_sl = xtw_sb[:L, k * XTW + DSH : (k + 1) * XTW]

                # contributions, kc-major: c[s, kc*DSH + d] = x[s,d] * tw[s,kc]
                # one rep-16 tile on DVE feeds both DVE and GPSIMD multiplies
                rep16 = repp.tile([128, KC * 16], BF16, name="rep16")
                r16v = rep16.rearrange("p (a c) -> p a c", c=16)
                nc.vector.tensor_copy(r16v[:L, :, 0:1], tw_sl[:, :, None])
                w = 1
                while w < 16:
                    nc.vector.tensor_copy(r16v[:L, :, w : 2 * w], r16v[:L, :, 0:w])
                    w *= 2
                if k == 0:
                    csb[0] = cp.tile([128, NKC], BF16, name="c_sb")
                c_sb = csb[k % 2]
                for a0, a1 in DVE_CHUNKS:
                    build(nc.vector, c_sb, L, a0, a1, x_sl, rep16)
                build(nc.gpsimd, c_sb, L, DVE_KC, KC, x_sl, rep16)
                emit_gps_slabs()
                if k + 1 < NBLK:
                    csb[(k + 1) % 2] = cp.tile([128, NKC], BF16, name="c_sb")
                    if k > 0:
                        carry_dma(k + 1)

                if k == 0:
                    carry_dma(0)
                    # upfront carry prefix sums (PE deltas + DVE adds); these
                    # trail warmup on PE and block 0's build on DVE, and all
                    # later carry rows only depend on them
                    for j in range(NBLK - 1):
                        Lj = min(BLK, T - j * BLK)
                        delta = pmain.tile([128, 2048], F32, name="pg")
                        nc.tensor.matmul(
                            delta[:KC, :DSH],
                            lhsT=xtw_sb[:Lj, j * XTW + DSH : (j + 1) * XTW],
                            rhs=xtw_sb[:Lj, j * XTW : j * XTW + DSH],
                            start=True, stop=True,
                        )
                        nc.vector.tensor_add(
                            carry_sb[:, (j + 1) * DSH : (j + 2) * DSH],
                            carry_sb[:, j * DSH : (j + 1) * DSH],
                            delta[:KC, :DSH],
                        )

                # full 128-column stationary (cols >= L are zero-padded in
                # utri) so walrus enables FWL on the LDWEIGHTS
                lhsT = utri_sb[: L + 1, 128 * k : 128 * (k + 1)]
                ogb[k % 3] = outp.tile([128, NKC], BF16, name="og")
                og = ogb[k % 3]
                for gi, (j0, gn) in enumerate(_EVICT_GROUPS):
                    pg = pmain.tile([128, 2048], F32, name="pg")
                    for jj in range(gn):
                        j = j0 + jj
                        nc.tensor.matmul(
                            pg[:, jj * 512 : (jj + 1) * 512],
                            lhsT=lhsT,
                            rhs=c_sb[: L + 1, j * 512 : (j + 1) * 512],
                            start=True, stop=True,
                        )
                    col = j0 * 512
                    if gi < DVE_EG:
                        nc.vector.tensor_copy(
                            og[:L, col : col + gn * 512], pg[:L, : gn * 512]
                        )
                    else:
                        nc.scalar.copy(
                            og[:L, col : col + gn * 512], pg[:L, : gn * 512]
                        )
                if k == 0:
                    carry_dma(1)
                store_pending = (k, t0, L)
                # stores issue immediately: sync/scalar queues have nothing
                # else pending, and ACT's own evicts precede its slabs so the
                # sem waits never block real work
                emit_stores()
            emit_gps_slabs()
    nc.compile()
    return nc


def kernel(**inputs) -> np.ndarray:
    global LAST_RESULTS
    x = np.asarray(inputs["x"])                       # (4,1024,512) bf16
    tw = np.asarray(inputs["twiddles"])               # (1024,32,2) bf16
    pos = np.asarray(inputs["pos_norm"])              # (1024,) bf16

    tw2 = np.ascontiguousarray(tw.reshape(T, KC))
    utri = _build_utri(pos)

    in_maps = []
    for core in range(8):
        b, dh = core // 2, core % 2
        xtw = np.concatenate(
            [x[b, :, dh * DSH : (dh + 1) * DSH], tw2], axis=1
        )
        in_maps.append({"xtw": np.ascontiguousarray(xtw), "utri": utri})

    nc = _build_program()
    res = run_bass_kernel_spmd(nc, in_maps, core_ids=list(range(8)))
    LAST_RESULTS = res

    out = np.empty((B, T, D, KC // 2, 2), dtype=x.dtype)
    for core in range(8):
        b, dh = core // 2, core % 2
        o = np.asarray(res.results[core]["out_shard"])  # (T, NKC) kc-major
        o = o.reshape(T, KC, DSH).transpose(0, 2, 1)    # -> (T, DSH, KC)
        out[b, :, dh * DSH : (dh + 1) * DSH, :, :] = o.reshape(T, DSH, KC // 2, 2)
    return out


if __name__ == "__main__":
    rng = np.random.default_rng(0)
    demo = {
        "x": rng.standard_normal((B, T, D), np.float32).astype(ml_dtypes.bfloat16),
        "twiddles": rng.standard_normal((T, KC // 2, 2), np.float32).astype(
            ml_dtypes.bfloat16
        ),
        "pos_norm": (1.0 / np.sqrt(np.arange(1, T + 1, dtype=np.float32))).astype(
            ml_dtypes.bfloat16
        ),
    }
    print(kernel(**demo).shape)
